# revision 8
# baseline (speedup 1.0000x reference)
"""Trainium2 Bass kernel for nn_CrossTransformerBlock (sparse kNN cross-attention).

Shapes (hardcoded): B=4, NQ=4096, N=2048, DIM=128, DG=256, DI=256, K=16.
Sharding: 8 cores = (batch b, query-half h); each core handles 2048 queries
against its batch's 2048 points.

Per-core pipeline, software-pipelined one block ahead:
  prep:  PE projects k_all/v_all (points @ W), g_all = W_d1@xyz, u = W_d1@xyz_q+b;
         PE-transposes them into a row-major DRAM table [2048, 384] f16 (k|v|g).
  topk:  PE computes s = 2 q.x - |x|^2 (float32r) in 1024-col pieces -> PSUM;
         DVE ORs the chunk-local column index into the mantissa low 7 bits
         (s|iota), then top-8 per 128-chunk (max8) -> 128 candidates; merge via
         max8 + match_replace -> top-16 packed; idx = (val & 0x7F) + chunk*128.
  gather: a selector matmul (E[q',p]=1[q'%16==p%16]) rearranges per-query idx
         into the wrapped int16 layout; 4 sub-dma_gathers (1024 idx each,
         transpose=True) pull neighbor rows feature-major into
         gath4 [128f, 4 groups, 3 slabs, 1024] -- sub-splitting bounds how long
         each GpSimd instruction holds the shared SBUF port pair, so DVE f16
         (2-port perf mode) ops can interleave.
  mlp:   h=relu(u-g) on DVE; the two wd2-consumers are PE-folded so `pos` never
         materializes: g1pre = wd2 h - k (+bias b_d2+q_attn via ACT),
         vpos = wd2 h + v (+b_d2); gamma MLP on PE with ACT Relu/Exp+bias on
         PSUM evacuation; exp and exp*vpos share one tile so the softmax
         denominator + weighted-sum trees run merged; global slot folded in as
         per-batch constants; 1/x via reciprocal_approx_fast.
"""

import numpy as np

import concourse.bass as bass
import concourse.bacc as bacc
import concourse.mybir as mybir
from concourse.tile import TileContext
from concourse.bass_utils import run_bass_kernel_spmd

F32 = mybir.dt.float32
F32R = mybir.dt.float32r
F16 = mybir.dt.float16
U32 = mybir.dt.uint32
I32 = mybir.dt.int32
I16 = mybir.dt.int16
ALU = mybir.AluOpType
ACTF = mybir.ActivationFunctionType

B, NQ, N, DIM, DG, DI, K = 4, 4096, 2048, 128, 256, 256, 16
NQC = 2048          # queries per core
QTILE = 128         # topk tile (queries on partitions)
NTILES = NQC // QTILE
QBLK = 256          # gather/MLP block
NBLK = NQC // QBLK
CHUNK = 128         # topk candidate chunk (top-8 per chunk)
NCHUNK = N // CHUNK
PIECE = 1024        # distance-psum evacuation piece
ROWF = 3 * DIM      # table row features (k|v|g)
NSUB = 4            # sub-gathers per block
SUBI = 4096 // NSUB

_CACHE = {}


def _build():
    nc = bacc.Bacc("TRN2", target_bir_lowering=False, debug=False, num_devices=8)

    # ---- external inputs (per core) ----
    qx4 = nc.dram_tensor("qx4", [4, NQC], F32, kind="ExternalInput")
    xt4 = nc.dram_tensor("xt4", [4, N], F32, kind="ExternalInput")
    ptsT = nc.dram_tensor("ptsT", [128, 2 * N], F16, kind="ExternalInput")
    xyzq4 = nc.dram_tensor("xyzq4", [4, NQC], F16, kind="ExternalInput")
    xyzn4 = nc.dram_tensor("xyzn4", [4, N], F16, kind="ExternalInput")
    wk_l = nc.dram_tensor("wk_l", [128, 2 * DIM], F16, kind="ExternalInput")
    wv_l = nc.dram_tensor("wv_l", [128, 2 * DIM], F16, kind="ExternalInput")
    wd1_l = nc.dram_tensor("wd1_l", [4, DIM], F16, kind="ExternalInput")
    wd2_l = nc.dram_tensor("wd2_l", [DIM, DIM], F16, kind="ExternalInput")
    wg1_l = nc.dram_tensor("wg1_l", [DIM, DIM], F16, kind="ExternalInput")
    wg2_l = nc.dram_tensor("wg2_l", [DIM, DIM], F16, kind="ExternalInput")
    # per-partition column vectors [128, 1] f32
    colv = nc.dram_tensor("colv", [DIM, 7], F32, kind="ExternalInput")
    # constants
    esel = nc.dram_tensor("esel", [128, 128], F16, kind="ExternalInput")
    masks = nc.dram_tensor("masks", [128, 2 * 256], F16, kind="ExternalInput")
    ident16 = nc.dram_tensor("ident16", [128, 128], F16, kind="ExternalInput")
    nident16 = nc.dram_tensor("nident16", [128, 128], F16, kind="ExternalInput")
    ident32 = nc.dram_tensor("ident32", [128, 128], F32, kind="ExternalInput")

    out = nc.dram_tensor("out", [NQC, DIM], F32, kind="ExternalOutput")

    with TileContext(nc) as tc:
        with tc.tile_pool(name="const", bufs=1) as cpool, \
             tc.tile_pool(name="prep", bufs=1) as prep, \
             tc.tile_pool(name="work", bufs=2) as work, \
             tc.tile_pool(name="wifp", bufs=4) as wifp, \
             tc.tile_pool(name="mlp", bufs=1) as mlp, \
             tc.tile_pool(name="psd", bufs=2, space="PSUM") as psd, \
             tc.tile_pool(name="psm", bufs=2, space="PSUM") as psm, \
             tc.tile_pool(name="pss", bufs=2, space="PSUM") as pss, \
             tc.tile_pool(name="dram", bufs=1, space="DRAM") as dpool:

            # ---------- load constants / operands ----------
            qx4_s = cpool.tile([4, NQC], F32)
            xt4_s = cpool.tile([4, N], F32)
            nc.sync.dma_start(qx4_s[:], qx4[:])
            nc.sync.dma_start(xt4_s[:], xt4[:])
            xyzq4_s = cpool.tile([4, NQC], F16)
            xyzn4_s = cpool.tile([4, N], F16)
            nc.sync.dma_start(xyzq4_s[:], xyzq4[:])
            nc.sync.dma_start(xyzn4_s[:], xyzn4[:])
            pts_s = cpool.tile([128, 2 * N], F16)
            nc.sync.dma_start(pts_s[:], ptsT[:])
            wk_s = cpool.tile([128, 2 * DIM], F16)
            wv_s = cpool.tile([128, 2 * DIM], F16)
            nc.sync.dma_start(wk_s[:], wk_l[:])
            nc.sync.dma_start(wv_s[:], wv_l[:])
            wd1_s = cpool.tile([4, DIM], F16)
            nc.sync.dma_start(wd1_s[:], wd1_l[:])
            wd2_s = cpool.tile([DIM, DIM], F16)
            wg1_s = cpool.tile([DIM, DIM], F16)
            wg2_s = cpool.tile([DIM, DIM], F16)
            nc.sync.dma_start(wd2_s[:], wd2_l[:])
            nc.sync.dma_start(wg1_s[:], wg1_l[:])
            nc.sync.dma_start(wg2_s[:], wg2_l[:])
            colv_s = cpool.tile([DIM, 7], F32)
            nc.sync.dma_start(colv_s[:], colv[:])
            b_d2 = colv_s[:, 0:1]
            b_g1 = colv_s[:, 1:2]
            b_g2 = colv_s[:, 2:3]
            eg = colv_s[:, 4:5]
            egv = colv_s[:, 5:6]
            bq = colv_s[:, 6:7]          # b_d2 + q_attn
            esel_s = cpool.tile([128, 128], F16)
            nc.sync.dma_start(esel_s[:], esel[:])
            masks_s = cpool.tile([128, 2 * 256], F16)
            nc.sync.dma_start(masks_s[:], masks[:])
            id16 = cpool.tile([128, 128], F16)
            nid16 = cpool.tile([128, 128], F16)
            id32 = cpool.tile([128, 128], F32)
            nc.sync.dma_start(id16[:], ident16[:])
            nc.sync.dma_start(nid16[:], nident16[:])
            nc.sync.dma_start(id32[:], ident32[:])

            # chunk-local column index 0..CHUNK-1 repeated; pattern repeats
            # every PIECE cols
            iota = cpool.tile([128, PIECE], I32)
            nc.gpsimd.iota(iota[:], pattern=[[0, PIECE // CHUNK], [1, CHUNK]],
                           base=0, channel_multiplier=0)
            # integer constants as per-partition columns (bitvec ALU ops
            # reject float immediates)
            bitc = cpool.tile([128, 4], U32)
            nc.vector.memset(bitc[:, 0:1], 0xFFFFFF80)
            nc.vector.memset(bitc[:, 1:2], 0x78)
            nc.vector.memset(bitc[:, 2:3], 4)
            nc.vector.memset(bitc[:, 3:4], 0x7F)

            # ---------- prep projections (feature-major) ----------
            kT = prep.tile([128, N], F16)
            vT = prep.tile([128, N], F16)
            gT = prep.tile([128, N], F16)
            uT = prep.tile([128, NQC], F16)
            for col in range(4):
                cs = slice(col * 512, (col + 1) * 512)
                acc_k = psm.tile([128, 512], F32, tag="mm")
                nc.tensor.matmul(acc_k[:], wk_s[:, 0:DIM],
                                 pts_s[:, col * 512:(col + 1) * 512],
                                 start=True, stop=False)
                nc.tensor.matmul(acc_k[:], wk_s[:, DIM:2 * DIM],
                                 pts_s[:, N + col * 512:N + (col + 1) * 512],
                                 start=False, stop=True)
                nc.scalar.copy(kT[:, cs], acc_k[:])
                acc_v = psm.tile([128, 512], F32, tag="mm")
                nc.tensor.matmul(acc_v[:], wv_s[:, 0:DIM],
                                 pts_s[:, col * 512:(col + 1) * 512],
                                 start=True, stop=False)
                nc.tensor.matmul(acc_v[:], wv_s[:, DIM:2 * DIM],
                                 pts_s[:, N + col * 512:N + (col + 1) * 512],
                                 start=False, stop=True)
                nc.scalar.copy(vT[:, cs], acc_v[:])
                acc_g = psm.tile([128, 512], F32, tag="mm")
                nc.tensor.matmul(acc_g[:], wd1_s[:], xyzn4_s[:, cs], start=True, stop=True)
                nc.scalar.copy(gT[:, cs], acc_g[:])
                acc_u = psm.tile([128, 512], F32, tag="mm")
                nc.tensor.matmul(acc_u[:], wd1_s[:], xyzq4_s[:, cs], start=True, stop=True)
                nc.scalar.copy(uT[:, cs], acc_u[:])

            # ---------- build DRAM gather table [N, 384] f16 ----------
            table = dpool.tile([N, ROWF], F16)
            for c in range(N // 128):
                rs = slice(c * 128, (c + 1) * 128)
                row_sb = work.tile([128, ROWF], F16, tag="rowsb")
                for j, src in enumerate((kT, vT, gT)):
                    pt = pss.tile([128, 128], F16, tag="small")
                    nc.tensor.transpose(pt[:], src[:, rs], id16[:])
                    nc.scalar.copy(row_sb[:, j * 128:(j + 1) * 128], pt[:])
                nc.sync.dma_start(table[rs, :], row_sb[:])

            # ---------- per-stage emitters ----------
            def emit_topk(t):
                """Top-16 for query tile t -> packed ids as f16 [128, 16]."""
                qs = slice(t * QTILE, (t + 1) * QTILE)
                cand = work.tile([128, 8 * NCHUNK], F32, tag="cand")
                for pc in range(N // PIECE):
                    sps = psd.tile([128, PIECE], F32, tag="dist")
                    for col in range(PIECE // 512):
                        cs = slice(pc * PIECE + col * 512,
                                   pc * PIECE + (col + 1) * 512)
                        nc.tensor.matmul(sps[:, col * 512:(col + 1) * 512],
                                         qx4_s[:, qs], xt4_s[:, cs],
                                         start=True, stop=True)
                    spk = work.tile([128, PIECE], U32, tag="spk")
                    nc.vector.scalar_tensor_tensor(
                        spk[:], sps[:].bitcast(U32), bitc[:, 0:1],
                        iota[:].bitcast(U32), ALU.bitwise_and, ALU.bitwise_or)
                    spkf = spk[:].bitcast(F32)
                    for c in range(PIECE // CHUNK):
                        cg = pc * (PIECE // CHUNK) + c
                        nc.vector.max(cand[:, cg * 8:(cg + 1) * 8],
                                      spkf[:, c * CHUNK:(c + 1) * CHUNK])
                winners = work.tile([128, 16], F32, tag="win")
                pos = work.tile([128, 16], U32, tag="pos")
                nc.vector.max(winners[:, 0:8], cand[:])
                nc.vector.max_index(pos[:, 0:8], winners[:, 0:8], cand[:])
                nc.vector.match_replace(cand[:], winners[:, 0:8], cand[:], -3e38)
                nc.vector.max(winners[:, 8:16], cand[:])
                nc.vector.max_index(pos[:, 8:16], winners[:, 8:16], cand[:])
                # global idx = (packed & 0x7F) + (pos//8)*CHUNK
                base = work.tile([128, 16], U32, tag="base")
                nc.vector.tensor_scalar(base[:], pos[:], bitc[:, 1:2],
                                        bitc[:, 2:3], ALU.bitwise_and,
                                        ALU.logical_shift_left)
                wid = work.tile([128, 16], U32, tag="wid")
                nc.vector.tensor_scalar(wid[:], winners[:].bitcast(U32),
                                        bitc[:, 3:4], None, ALU.bitwise_and)
                nc.vector.tensor_tensor(wid[:], wid[:], base[:], ALU.add)
                wif = wifp.tile([128, 16], F16, tag="wif")
                nc.vector.tensor_copy(wif[:], wid[:])
                return wif

            def emit_gather(gb, wif_list):
                """Selector matmul -> wrapped idx; 4 sub-gathers for block gb."""
                psel = pss.tile([128, 256], F32, tag="small")
                for t2 in range(2):
                    rhs = work.tile([128, 256], F16, tag="rhs")
                    nc.vector.tensor_tensor(
                        rhs[:].rearrange("p (a b) -> p a b", a=16),
                        wif_list[t2][:].unsqueeze(2).broadcast_to((128, 16, 16)),
                        masks_s[:, t2 * 256:(t2 + 1) * 256].rearrange("p (a b) -> p a b", a=16),
                        ALU.mult)
                    nc.tensor.matmul(psel[:], esel_s[:], rhs[:],
                                     start=(t2 == 0), stop=(t2 == 1))
                idxs = work.tile([128, 256], I16, tag="idxs")
                nc.scalar.copy(idxs[:], psel[:])
                gath = work.tile([128, NSUB, 3, SUBI], F16, tag="gath")
                for i in range(NSUB):
                    nc.gpsimd.dma_gather(
                        gath[:, i, :, :], table[:],
                        idxs[:, i * (SUBI // 16):(i + 1) * (SUBI // 16)],
                        num_idxs=SUBI, num_idxs_reg=SUBI,
                        elem_size=ROWF, transpose=True, single_packet=False)
                return gath

            def emit_mlp(gb, gath):
                NG = 16 // NSUB     # neighbors per sub-gather group
                # 3-free-dim views [p][group][nbr-in-group][query]
                def slab(s):
                    return gath[:, :, s, :].rearrange("p g (a b) -> p g a b", a=NG)
                k4, v4, g4 = slab(0), slab(1), slab(2)
                ub = uT[:, gb * QBLK:(gb + 1) * QBLK].unsqueeze(1).unsqueeze(1) \
                    .broadcast_to((128, NSUB, NG, QBLK))

                # --- h = relu(u - g) (DVE) ---
                hpre = mlp.tile([128, 4096], F16, tag="ma")
                h4 = hpre[:].rearrange("p (g a b) -> p g a b", g=NSUB, a=NG)
                nc.vector.tensor_tensor(h4, ub, g4, ALU.subtract)
                nc.vector.tensor_scalar_max(hpre[:], hpre[:], 0.0)

                # --- g1pre = wd2 h - k + (b_d2 + q_attn); vpos = wd2 h + v + b_d2
                # (PE-folded; `pos` never materializes) ---
                g1pre = mlp.tile([128, 4096], F16, tag="mb")
                vpos = mlp.tile([128, 4096], F16, tag="mc")
                for col in range(8):
                    cs = slice(col * 512, (col + 1) * 512)
                    grp, off = col // 2, (col % 2) * 512
                    pm = psm.tile([128, 512], F32, tag="mm")
                    nc.tensor.matmul(pm[:], wd2_s[:], hpre[:, cs], start=True, stop=False)
                    nc.tensor.matmul(pm[:], nid16[:], gath[:, grp, 0, off:off + 512],
                                     start=False, stop=True)
                    nc.scalar.add(g1pre[:, cs], pm[:], bq)
                    pm2 = psm.tile([128, 512], F32, tag="mm")
                    nc.tensor.matmul(pm2[:], wd2_s[:], hpre[:, cs], start=True, stop=False)
                    nc.tensor.matmul(pm2[:], id16[:], gath[:, grp, 1, off:off + 512],
                                     start=False, stop=True)
                    nc.scalar.add(vpos[:, cs], pm2[:], b_d2)

                # --- g1 = relu(W_g1 @ g1pre + b_g1) ---
                g1 = mlp.tile([128, 4096], F16, tag="ma")
                for col in range(8):
                    cs = slice(col * 512, (col + 1) * 512)
                    pm = psm.tile([128, 512], F32, tag="mm")
                    nc.tensor.matmul(pm[:], wg1_s[:], g1pre[:, cs], start=True, stop=True)
                    nc.scalar.activation(g1[:, cs], pm[:], ACTF.Relu, bias=b_g1)

                # --- we = [exp(W_g2 @ g1 + b_g2) | expt * vpos] (one tile so the
                # two 16-neighbor reduction trees run merged) ---
                we = mlp.tile([128, 2 * 4096], F16, tag="md")
                expt = we[:, 0:4096]
                for col in range(8):
                    cs = slice(col * 512, (col + 1) * 512)
                    pm = psm.tile([128, 512], F32, tag="mm")
                    nc.tensor.matmul(pm[:], wg2_s[:], g1[:, cs], start=True, stop=True)
                    nc.scalar.activation(we[:, cs], pm[:], ACTF.Exp, bias=b_g2)
                nc.vector.tensor_tensor(we[:, 4096:8192], we[:, 0:4096],
                                        vpos[:], ALU.mult)

                # --- merged esum/wsum trees (f16 halves, f32 final) ---
                we3 = we[:].rearrange("p (c a b) -> p c a b", c=2, a=16)
                t8 = mlp.tile([128, 2, 8, QBLK], F16, tag="t8")
                nc.vector.tensor_tensor(t8[:], we3[:, :, 0:8, :], we3[:, :, 8:16, :], ALU.add)
                t4 = mlp.tile([128, 2, 4, QBLK], F16, tag="t4")
                nc.vector.tensor_tensor(t4[:], t8[:, :, 0:4, :], t8[:, :, 4:8, :], ALU.add)
                t2 = mlp.tile([128, 2, 2, QBLK], F16, tag="t2")
                nc.vector.tensor_tensor(t2[:], t4[:, :, 0:2, :], t4[:, :, 2:4, :], ALU.add)
                sums = mlp.tile([128, 2, QBLK], F32, tag="es")
                nc.vector.tensor_tensor(sums[:], t2[:, :, 0, :], t2[:, :, 1, :], ALU.add)
                esum = sums[:, 0, :]
                wsum = sums[:, 1, :]

                # --- fold global slot; normalize ---
                nc.vector.tensor_scalar_add(esum, esum, eg)
                rcp = mlp.tile([128, QBLK], F32, tag="rc")
                nc.vector.reciprocal(rcp[:], esum)
                res = mlp.tile([128, QBLK], F32, tag="res")
                nc.vector.scalar_tensor_tensor(res[:], wsum, egv, rcp[:],
                                               ALU.add, ALU.mult)

                # --- transpose out and store ---
                for t2i in range(2):
                    po = pss.tile([128, 128], F32, tag="small")
                    nc.tensor.transpose(po[:], res[:, t2i * 128:(t2i + 1) * 128], id32[:])
                    osb = work.tile([128, 128], F32, tag="osb")
                    nc.scalar.copy(osb[:], po[:])
                    nc.sync.dma_start(
                        out[gb * QBLK + t2i * 128: gb * QBLK + (t2i + 1) * 128, :],
                        osb[:])

            # ---------- main loop, software-pipelined ----------
            # iteration i: gather(i) (idxs from topk emitted last iteration),
            # topk(i+1) (overlaps gather(i) on DVE -- 1-port f32 ops),
            # mlp(i-1) (f16 ops wedge into sub-gather gaps).
            wif_q = [emit_topk(0), emit_topk(1)]
            gath_q = []
            for i in range(NBLK):
                gath_q.append(emit_gather(i, wif_q))
                if i + 1 < NBLK:
                    wif_q = [emit_topk(2 * (i + 1)), emit_topk(2 * (i + 1) + 1)]
                if i >= 1:
                    emit_mlp(i - 1, gath_q.pop(0))
            emit_mlp(NBLK - 1, gath_q.pop(0))

    nc.compile()
    return nc


def _host_prep(inputs):
    """Build the 8 per-core input maps from full inputs (layout prep only)."""
    xyz_q = np.asarray(inputs["xyz_q"], np.float32)
    lat_rep = np.asarray(inputs["lat_rep"], np.float32)
    xyz = np.asarray(inputs["xyz"], np.float32)
    points = np.asarray(inputs["points"], np.float32)
    W_d1 = np.asarray(inputs["W_d1"], np.float32); b_d1 = np.asarray(inputs["b_d1"], np.float32)
    W_d2 = np.asarray(inputs["W_d2"], np.float32); b_d2 = np.asarray(inputs["b_d2"], np.float32)
    W_g1 = np.asarray(inputs["W_g1"], np.float32); b_g1 = np.asarray(inputs["b_g1"], np.float32)
    W_g2 = np.asarray(inputs["W_g2"], np.float32); b_g2 = np.asarray(inputs["b_g2"], np.float32)
    W_kg = np.asarray(inputs["W_kg"], np.float32)
    W_vg = np.asarray(inputs["W_vg"], np.float32)
    W_q = np.asarray(inputs["W_q"], np.float32)
    W_k = np.asarray(inputs["W_k"], np.float32)
    W_v = np.asarray(inputs["W_v"], np.float32)

    # per-batch global-slot constants
    q_attn = lat_rep @ W_q.T                      # [B, DIM]
    k_g = lat_rep @ W_kg.T
    v_g = lat_rep @ W_vg.T
    tg = q_attn - k_g
    g1g = np.maximum(tg @ W_g1.T + b_g1, 0.0)
    logit_g = g1g @ W_g2.T + b_g2
    exp_g = np.exp(logit_g)                       # [B, DIM]
    egv = exp_g * v_g

    # constants
    qp = np.arange(128)
    esel = (qp[:, None] % 16 == qp[None, :] % 16).astype(np.float16)  # [q',p]
    masks = np.zeros((2, 128, 256), np.float16)
    g_of = qp // 16                               # q' // 16 in 0..7
    for t in range(2):
        for nb in range(16):
            for g in range(16):
                masks[t, :, nb * 16 + g] = (g_of == (g - t * 8)).astype(np.float16)
    ident16 = np.eye(128, dtype=np.float16)
    nident16 = -np.eye(128, dtype=np.float16)
    ident32 = np.eye(128, dtype=np.float32)

    wd1_l = np.concatenate([W_d1.T, b_d1[None, :]], axis=0).astype(np.float16)  # [4,128]

    maps = []
    for core in range(8):
        b, h = core // 2, core % 2
        qsl = slice(h * NQC, (h + 1) * NQC)
        xq = xyz_q[b, qsl]                        # [2048, 3]
        xn = xyz[b]                               # [2048, 3]
        qx4 = np.concatenate([2.0 * xq.T, np.ones((1, NQC), np.float32)], axis=0)
        xt4 = np.concatenate([xn.T, -np.sum(xn * xn, axis=1)[None, :]], axis=0)
        xyzq4 = np.concatenate([xq.T, np.ones((1, NQC), np.float32)], axis=0).astype(np.float16)
        xyzn4 = np.concatenate([xn.T, np.zeros((1, N), np.float32)], axis=0).astype(np.float16)
        pT = points[b].T.astype(np.float16)          # [256, N]
        ptsT = np.concatenate([pT[0:128], pT[128:256]], axis=1)  # [128, 2N]
        colv = np.stack([b_d2, b_g1, b_g2, q_attn[b], exp_g[b], egv[b],
                         b_d2 + q_attn[b]],
                        axis=1).astype(np.float32)
        maps.append({
            "qx4": np.ascontiguousarray(qx4, np.float32),
            "xt4": np.ascontiguousarray(xt4, np.float32),
            "ptsT": np.ascontiguousarray(ptsT),
            "xyzq4": np.ascontiguousarray(xyzq4),
            "xyzn4": np.ascontiguousarray(xyzn4),
            "wk_l": np.ascontiguousarray(np.concatenate(
                [W_k.T[0:128], W_k.T[128:256]], axis=1).astype(np.float16)),
            "wv_l": np.ascontiguousarray(np.concatenate(
                [W_v.T[0:128], W_v.T[128:256]], axis=1).astype(np.float16)),
            "wd1_l": np.ascontiguousarray(wd1_l),
            "wd2_l": np.ascontiguousarray(W_d2.T.astype(np.float16)),
            "wg1_l": np.ascontiguousarray(W_g1.T.astype(np.float16)),
            "wg2_l": np.ascontiguousarray(W_g2.T.astype(np.float16)),
            "colv": np.ascontiguousarray(colv),
            "esel": np.ascontiguousarray(esel),
            "masks": np.ascontiguousarray(
                np.concatenate([masks[0], masks[1]], axis=1)),
            "ident16": ident16,
            "nident16": nident16,
            "ident32": ident32,
        })
    return maps


def kernel(**inputs):
    if "nc" not in _CACHE:
        _CACHE["nc"] = _build()
    nc = _CACHE["nc"]
    maps = _host_prep(inputs)
    res = run_bass_kernel_spmd(nc, maps, core_ids=list(range(8)))
    _CACHE["last_results"] = res
    out = np.empty((B, NQ, DIM), np.float32)
    for core in range(8):
        b, h = core // 2, core % 2
        out[b, h * NQC:(h + 1) * NQC, :] = res.results[core]["out"]
    return out


# revision 9
# speedup vs baseline: 1.2445x; 1.2445x over previous
"""Trainium2 Bass kernel for nn_CrossTransformerBlock (sparse kNN cross-attention).

Shapes (hardcoded): B=4, NQ=4096, N=2048, DIM=128, DG=256, DI=256, K=16.
Sharding: 8 cores = (batch b, query-half h); each core handles 2048 queries
against its batch's 2048 points.

Per-core pipeline, software-pipelined one block ahead:
  prep:  PE projects k_all/v_all (points @ W), g_all = W_d1@xyz, u = W_d1@xyz_q+b;
         PE-transposes them into a row-major DRAM table [2048, 384] f16 (k|v|g).
  topk:  PE computes s = 2 q.x - |x|^2 (float32r) in 1024-col pieces -> PSUM;
         DVE ORs the chunk-local column index into the mantissa low 7 bits
         (s|iota), then top-8 per 128-chunk (max8) -> 128 candidates; merge via
         max8 + match_replace -> top-16 packed; idx = (val & 0x7F) + chunk*128.
  gather: a selector matmul (E[q',p]=1[q'%16==p%16]) rearranges per-query idx
         into the wrapped int16 layout; 4 sub-dma_gathers (1024 idx each,
         transpose=True) pull neighbor rows feature-major into
         gath4 [128f, 4 groups, 3 slabs, 1024] -- sub-splitting bounds how long
         each GpSimd instruction holds the shared SBUF port pair, so DVE f16
         (2-port perf mode) ops can interleave.
  mlp:   h=relu(u-g) on DVE; the two wd2-consumers are PE-folded so `pos` never
         materializes: g1pre = wd2 h - k (+bias b_d2+q_attn via ACT),
         vpos = wd2 h + v (+b_d2); gamma MLP on PE with ACT Relu/Exp+bias on
         PSUM evacuation; exp and exp*vpos share one tile so the softmax
         denominator + weighted-sum trees run merged; global slot folded in as
         per-batch constants; 1/x via reciprocal_approx_fast.
"""

import numpy as np

import concourse.bass as bass
import concourse.bacc as bacc
import concourse.mybir as mybir
from concourse.tile import TileContext
from concourse.bass_utils import run_bass_kernel_spmd

F32 = mybir.dt.float32
F32R = mybir.dt.float32r
F16 = mybir.dt.float16
U32 = mybir.dt.uint32
I32 = mybir.dt.int32
I16 = mybir.dt.int16
ALU = mybir.AluOpType
ACTF = mybir.ActivationFunctionType

B, NQ, N, DIM, DG, DI, K = 4, 4096, 2048, 128, 256, 256, 16
NQC = 2048          # queries per core
QTILE = 128         # topk tile (queries on partitions)
NTILES = NQC // QTILE
QBLK = 256          # gather/MLP block
NBLK = NQC // QBLK
CHUNK = 128         # topk candidate chunk (top-8 per chunk)
NCHUNK = N // CHUNK
PIECE = 1024        # distance-psum evacuation piece
ROWF = 3 * DIM      # table row features (k|v|g)
NSUB = 4            # sub-gathers per block
SUBI = 4096 // NSUB

_CACHE = {}


def _build():
    nc = bacc.Bacc("TRN2", target_bir_lowering=False, debug=False, num_devices=8)

    # ---- external inputs (per core) ----
    qx4 = nc.dram_tensor("qx4", [4, NQC], F32, kind="ExternalInput")
    xt4 = nc.dram_tensor("xt4", [4, N], F32, kind="ExternalInput")
    ptsT = nc.dram_tensor("ptsT", [128, 2 * N], F16, kind="ExternalInput")
    xyzq4 = nc.dram_tensor("xyzq4", [4, NQC], F16, kind="ExternalInput")
    xyzn4 = nc.dram_tensor("xyzn4", [4, N], F16, kind="ExternalInput")
    wk_l = nc.dram_tensor("wk_l", [128, 2 * DIM], F16, kind="ExternalInput")
    wv_l = nc.dram_tensor("wv_l", [128, 2 * DIM], F16, kind="ExternalInput")
    wd1_l = nc.dram_tensor("wd1_l", [4, DIM], F16, kind="ExternalInput")
    wd2_l = nc.dram_tensor("wd2_l", [DIM, DIM], F16, kind="ExternalInput")
    wg1_l = nc.dram_tensor("wg1_l", [DIM, DIM], F16, kind="ExternalInput")
    wg2_l = nc.dram_tensor("wg2_l", [DIM, DIM], F16, kind="ExternalInput")
    # per-partition column vectors [128, 1] f32
    colv = nc.dram_tensor("colv", [DIM, 7], F32, kind="ExternalInput")
    # constants
    esel = nc.dram_tensor("esel", [128, 128], F16, kind="ExternalInput")
    masks = nc.dram_tensor("masks", [128, 2 * 256], F16, kind="ExternalInput")
    ident16 = nc.dram_tensor("ident16", [128, 128], F16, kind="ExternalInput")
    nident16 = nc.dram_tensor("nident16", [128, 128], F16, kind="ExternalInput")
    ident32 = nc.dram_tensor("ident32", [128, 128], F32, kind="ExternalInput")

    out = nc.dram_tensor("out", [NQC, DIM], F32, kind="ExternalOutput")

    with TileContext(nc) as tc:
        with tc.tile_pool(name="const", bufs=1) as cpool, \
             tc.tile_pool(name="prep", bufs=1) as prep, \
             tc.tile_pool(name="work", bufs=2) as work, \
             tc.tile_pool(name="wifp", bufs=4) as wifp, \
             tc.tile_pool(name="gpool", bufs=3) as gpool, \
             tc.tile_pool(name="mlp", bufs=1) as mlp, \
             tc.tile_pool(name="psd", bufs=2, space="PSUM") as psd, \
             tc.tile_pool(name="psm", bufs=2, space="PSUM") as psm, \
             tc.tile_pool(name="pss", bufs=2, space="PSUM") as pss, \
             tc.tile_pool(name="dram", bufs=1, space="DRAM") as dpool:

            # ---------- load constants / operands ----------
            qx4_s = cpool.tile([4, NQC], F32)
            xt4_s = cpool.tile([4, N], F32)
            nc.sync.dma_start(qx4_s[:], qx4[:])
            nc.sync.dma_start(xt4_s[:], xt4[:])
            xyzq4_s = cpool.tile([4, NQC], F16)
            xyzn4_s = cpool.tile([4, N], F16)
            nc.sync.dma_start(xyzq4_s[:], xyzq4[:])
            nc.sync.dma_start(xyzn4_s[:], xyzn4[:])
            pts_s = cpool.tile([128, 2 * N], F16)
            nc.sync.dma_start(pts_s[:], ptsT[:])
            wk_s = cpool.tile([128, 2 * DIM], F16)
            wv_s = cpool.tile([128, 2 * DIM], F16)
            nc.sync.dma_start(wk_s[:], wk_l[:])
            nc.sync.dma_start(wv_s[:], wv_l[:])
            wd1_s = cpool.tile([4, DIM], F16)
            nc.sync.dma_start(wd1_s[:], wd1_l[:])
            wd2_s = cpool.tile([DIM, DIM], F16)
            wg1_s = cpool.tile([DIM, DIM], F16)
            wg2_s = cpool.tile([DIM, DIM], F16)
            nc.sync.dma_start(wd2_s[:], wd2_l[:])
            nc.sync.dma_start(wg1_s[:], wg1_l[:])
            nc.sync.dma_start(wg2_s[:], wg2_l[:])
            colv_s = cpool.tile([DIM, 7], F32)
            nc.sync.dma_start(colv_s[:], colv[:])
            b_d2 = colv_s[:, 0:1]
            b_g1 = colv_s[:, 1:2]
            b_g2 = colv_s[:, 2:3]
            eg = colv_s[:, 4:5]
            egv = colv_s[:, 5:6]
            bq = colv_s[:, 6:7]          # b_d2 + q_attn
            esel_s = cpool.tile([128, 128], F16)
            nc.sync.dma_start(esel_s[:], esel[:])
            masks_s = cpool.tile([128, 2 * 256], F16)
            nc.sync.dma_start(masks_s[:], masks[:])
            id16 = cpool.tile([128, 128], F16)
            nid16 = cpool.tile([128, 128], F16)
            id32 = cpool.tile([128, 128], F32)
            nc.sync.dma_start(id16[:], ident16[:])
            nc.sync.dma_start(nid16[:], nident16[:])
            nc.sync.dma_start(id32[:], ident32[:])

            # chunk-local column index 0..CHUNK-1 repeated; pattern repeats
            # every PIECE cols
            iota = cpool.tile([128, PIECE], I32)
            nc.gpsimd.iota(iota[:], pattern=[[0, PIECE // CHUNK], [1, CHUNK]],
                           base=0, channel_multiplier=0)
            # integer constants as per-partition columns (bitvec ALU ops
            # reject float immediates)
            bitc = cpool.tile([128, 4], U32)
            nc.vector.memset(bitc[:, 0:1], 0xFFFFFF80)
            nc.vector.memset(bitc[:, 1:2], 0x78)
            nc.vector.memset(bitc[:, 2:3], 4)
            nc.vector.memset(bitc[:, 3:4], 0x7F)

            # ---------- prep projections (feature-major) ----------
            kT = prep.tile([128, N], F16)
            vT = prep.tile([128, N], F16)
            gT = prep.tile([128, N], F16)
            uT = prep.tile([128, NQC], F16)
            for col in range(4):
                cs = slice(col * 512, (col + 1) * 512)
                acc_k = psm.tile([128, 512], F32, tag="mm")
                nc.tensor.matmul(acc_k[:], wk_s[:, 0:DIM],
                                 pts_s[:, col * 512:(col + 1) * 512],
                                 start=True, stop=False)
                nc.tensor.matmul(acc_k[:], wk_s[:, DIM:2 * DIM],
                                 pts_s[:, N + col * 512:N + (col + 1) * 512],
                                 start=False, stop=True)
                nc.scalar.copy(kT[:, cs], acc_k[:])
                acc_v = psm.tile([128, 512], F32, tag="mm")
                nc.tensor.matmul(acc_v[:], wv_s[:, 0:DIM],
                                 pts_s[:, col * 512:(col + 1) * 512],
                                 start=True, stop=False)
                nc.tensor.matmul(acc_v[:], wv_s[:, DIM:2 * DIM],
                                 pts_s[:, N + col * 512:N + (col + 1) * 512],
                                 start=False, stop=True)
                nc.scalar.copy(vT[:, cs], acc_v[:])
                acc_g = psm.tile([128, 512], F32, tag="mm")
                nc.tensor.matmul(acc_g[:], wd1_s[:], xyzn4_s[:, cs], start=True, stop=True)
                nc.scalar.copy(gT[:, cs], acc_g[:])
                acc_u = psm.tile([128, 512], F32, tag="mm")
                nc.tensor.matmul(acc_u[:], wd1_s[:], xyzq4_s[:, cs], start=True, stop=True)
                nc.scalar.copy(uT[:, cs], acc_u[:])

            # ---------- DRAM gather table [N, 384] f16 (emitted later so the
            # PE reaches the first distance matmuls sooner) ----------
            table = dpool.tile([N, ROWF], F16)

            def emit_table():
                for c in range(N // 128):
                    rs = slice(c * 128, (c + 1) * 128)
                    row_sb = work.tile([128, ROWF], F16, tag="rowsb")
                    for j, srcT in enumerate((kT, vT, gT)):
                        pt = pss.tile([128, 128], F16, tag="small")
                        nc.tensor.transpose(pt[:], srcT[:, rs], id16[:])
                        nc.scalar.copy(row_sb[:, j * 128:(j + 1) * 128], pt[:])
                    nc.sync.dma_start(table[rs, :], row_sb[:])

            # ---------- per-stage emitters ----------
            def emit_topk(t):
                """Top-16 for query tile t -> packed ids as f16 [128, 16]."""
                qs = slice(t * QTILE, (t + 1) * QTILE)
                cand = work.tile([128, 8 * NCHUNK], F32, tag="cand")
                for pc in range(N // PIECE):
                    sps = psd.tile([128, PIECE], F32, tag="dist")
                    for col in range(PIECE // 512):
                        cs = slice(pc * PIECE + col * 512,
                                   pc * PIECE + (col + 1) * 512)
                        nc.tensor.matmul(sps[:, col * 512:(col + 1) * 512],
                                         qx4_s[:, qs], xt4_s[:, cs],
                                         start=True, stop=True)
                    spk = work.tile([128, PIECE], U32, tag="spk")
                    nc.vector.scalar_tensor_tensor(
                        spk[:], sps[:].bitcast(U32), bitc[:, 0:1],
                        iota[:].bitcast(U32), ALU.bitwise_and, ALU.bitwise_or)
                    spkf = spk[:].bitcast(F32)
                    for c in range(PIECE // CHUNK):
                        cg = pc * (PIECE // CHUNK) + c
                        nc.vector.max(cand[:, cg * 8:(cg + 1) * 8],
                                      spkf[:, c * CHUNK:(c + 1) * CHUNK])
                winners = work.tile([128, 16], F32, tag="win")
                pos = work.tile([128, 16], U32, tag="pos")
                nc.vector.max(winners[:, 0:8], cand[:])
                nc.vector.max_index(pos[:, 0:8], winners[:, 0:8], cand[:])
                nc.vector.match_replace(cand[:], winners[:, 0:8], cand[:], -3e38)
                nc.vector.max(winners[:, 8:16], cand[:])
                nc.vector.max_index(pos[:, 8:16], winners[:, 8:16], cand[:])
                # global idx = (packed & 0x7F) + (pos//8)*CHUNK
                base = work.tile([128, 16], U32, tag="base")
                nc.vector.tensor_scalar(base[:], pos[:], bitc[:, 1:2],
                                        bitc[:, 2:3], ALU.bitwise_and,
                                        ALU.logical_shift_left)
                wid = work.tile([128, 16], U32, tag="wid")
                nc.vector.tensor_scalar(wid[:], winners[:].bitcast(U32),
                                        bitc[:, 3:4], None, ALU.bitwise_and)
                nc.vector.tensor_tensor(wid[:], wid[:], base[:], ALU.add)
                wif = wifp.tile([128, 16], F16, tag="wif")
                nc.vector.tensor_copy(wif[:], wid[:])
                return wif

            def emit_gather(gb, wif_list):
                """Selector matmul -> wrapped idx; 4 sub-gathers for block gb."""
                psel = pss.tile([128, 256], F32, tag="small")
                for t2 in range(2):
                    rhs = work.tile([128, 256], F16, tag="rhs")
                    nc.vector.tensor_tensor(
                        rhs[:].rearrange("p (a b) -> p a b", a=16),
                        wif_list[t2][:].unsqueeze(2).broadcast_to((128, 16, 16)),
                        masks_s[:, t2 * 256:(t2 + 1) * 256].rearrange("p (a b) -> p a b", a=16),
                        ALU.mult)
                    nc.tensor.matmul(psel[:], esel_s[:], rhs[:],
                                     start=(t2 == 0), stop=(t2 == 1))
                idxs = gpool.tile([128, 256], I16, tag="idxs")
                nc.scalar.copy(idxs[:], psel[:])
                gath = gpool.tile([128, NSUB, 3, SUBI], F16, tag="gath")
                for i in range(NSUB):
                    nc.gpsimd.dma_gather(
                        gath[:, i, :, :], table[:],
                        idxs[:, i * (SUBI // 16):(i + 1) * (SUBI // 16)],
                        num_idxs=SUBI, num_idxs_reg=SUBI,
                        elem_size=ROWF, transpose=True, single_packet=False)
                return gath

            def emit_mlp(gb, gath):
                NG = 16 // NSUB     # neighbors per sub-gather group
                # 3-free-dim views [p][group][nbr-in-group][query]
                def slab(s):
                    return gath[:, :, s, :].rearrange("p g (a b) -> p g a b", a=NG)
                k4, v4, g4 = slab(0), slab(1), slab(2)
                ub = uT[:, gb * QBLK:(gb + 1) * QBLK].unsqueeze(1).unsqueeze(1) \
                    .broadcast_to((128, NSUB, NG, QBLK))

                # --- h = relu(u - g) (DVE) ---
                hpre = mlp.tile([128, 4096], F16, tag="ma")
                h4 = hpre[:].rearrange("p (g a b) -> p g a b", g=NSUB, a=NG)
                nc.vector.tensor_tensor(h4, ub, g4, ALU.subtract)
                nc.vector.tensor_scalar_max(hpre[:], hpre[:], 0.0)

                # --- g1pre = wd2 h - k + (b_d2 + q_attn); vpos = wd2 h + v + b_d2
                # (PE-folded; `pos` never materializes) ---
                g1pre = mlp.tile([128, 4096], F16, tag="mb")
                vpos = mlp.tile([128, 4096], F16, tag="mc")
                for col in range(8):
                    cs = slice(col * 512, (col + 1) * 512)
                    grp, off = col // 2, (col % 2) * 512
                    pm = psm.tile([128, 512], F32, tag="mm")
                    nc.tensor.matmul(pm[:], wd2_s[:], hpre[:, cs], start=True, stop=False)
                    nc.tensor.matmul(pm[:], nid16[:], gath[:, grp, 0, off:off + 512],
                                     start=False, stop=True)
                    nc.scalar.add(g1pre[:, cs], pm[:], bq)
                    pm2 = psm.tile([128, 512], F32, tag="mm")
                    nc.tensor.matmul(pm2[:], wd2_s[:], hpre[:, cs], start=True, stop=False)
                    nc.tensor.matmul(pm2[:], id16[:], gath[:, grp, 1, off:off + 512],
                                     start=False, stop=True)
                    nc.scalar.add(vpos[:, cs], pm2[:], b_d2)

                # --- g1 = relu(W_g1 @ g1pre + b_g1) ---
                g1 = mlp.tile([128, 4096], F16, tag="ma")
                for col in range(8):
                    cs = slice(col * 512, (col + 1) * 512)
                    pm = psm.tile([128, 512], F32, tag="mm")
                    nc.tensor.matmul(pm[:], wg1_s[:], g1pre[:, cs], start=True, stop=True)
                    nc.scalar.activation(g1[:, cs], pm[:], ACTF.Relu, bias=b_g1)

                # --- we = [exp(W_g2 @ g1 + b_g2) | expt * vpos] (one tile so the
                # two 16-neighbor reduction trees run merged) ---
                we = mlp.tile([128, 2 * 4096], F16, tag="md")
                expt = we[:, 0:4096]
                for col in range(8):
                    cs = slice(col * 512, (col + 1) * 512)
                    pm = psm.tile([128, 512], F32, tag="mm")
                    nc.tensor.matmul(pm[:], wg2_s[:], g1[:, cs], start=True, stop=True)
                    nc.scalar.activation(we[:, cs], pm[:], ACTF.Exp, bias=b_g2)
                nc.vector.tensor_tensor(we[:, 4096:8192], we[:, 0:4096],
                                        vpos[:], ALU.mult)

                # --- merged esum/wsum trees (f16 halves, f32 final) ---
                we3 = we[:].rearrange("p (c a b) -> p c a b", c=2, a=16)
                t8 = mlp.tile([128, 2, 8, QBLK], F16, tag="t8")
                nc.vector.tensor_tensor(t8[:], we3[:, :, 0:8, :], we3[:, :, 8:16, :], ALU.add)
                t4 = mlp.tile([128, 2, 4, QBLK], F16, tag="t4")
                nc.vector.tensor_tensor(t4[:], t8[:, :, 0:4, :], t8[:, :, 4:8, :], ALU.add)
                t2 = mlp.tile([128, 2, 2, QBLK], F16, tag="t2")
                nc.vector.tensor_tensor(t2[:], t4[:, :, 0:2, :], t4[:, :, 2:4, :], ALU.add)
                sums = mlp.tile([128, 2, QBLK], F32, tag="es")
                nc.vector.tensor_tensor(sums[:], t2[:, :, 0, :], t2[:, :, 1, :], ALU.add)
                esum = sums[:, 0, :]
                wsum = sums[:, 1, :]

                # --- fold global slot; normalize ---
                nc.vector.tensor_scalar_add(esum, esum, eg)
                rcp = mlp.tile([128, QBLK], F32, tag="rc")
                nc.vector.reciprocal_approx_fast(rcp[:], esum)
                res = mlp.tile([128, QBLK], F32, tag="res")
                nc.vector.scalar_tensor_tensor(res[:], wsum, egv, rcp[:],
                                               ALU.add, ALU.mult)

                # --- transpose out and store ---
                for t2i in range(2):
                    po = pss.tile([128, 128], F32, tag="small")
                    nc.tensor.transpose(po[:], res[:, t2i * 128:(t2i + 1) * 128], id32[:])
                    osb = work.tile([128, 128], F32, tag="osb")
                    nc.scalar.copy(osb[:], po[:])
                    nc.sync.dma_start(
                        out[gb * QBLK + t2i * 128: gb * QBLK + (t2i + 1) * 128, :],
                        osb[:])

            # ---------- main loop, software-pipelined two blocks deep ----------
            # topk+selector+gather run two blocks ahead of the MLP so the Pool
            # queue never starves; topk's f32 DVE ops overlap gathers (1-port),
            # while the MLP's f16 (2-port) ops wedge between sub-gathers.
            wif_q = [emit_topk(0), emit_topk(1)]
            emit_table()
            gath_q = [emit_gather(0, wif_q)]
            wif_q = [emit_topk(2), emit_topk(3)]
            gath_q.append(emit_gather(1, wif_q))
            for i in range(NBLK):
                if i + 2 < NBLK:
                    wif_q = [emit_topk(2 * (i + 2)), emit_topk(2 * (i + 2) + 1)]
                    gath_q.append(emit_gather(i + 2, wif_q))
                emit_mlp(i, gath_q.pop(0))

    nc.compile()
    return nc


def _host_prep(inputs):
    """Build the 8 per-core input maps from full inputs (layout prep only)."""
    xyz_q = np.asarray(inputs["xyz_q"], np.float32)
    lat_rep = np.asarray(inputs["lat_rep"], np.float32)
    xyz = np.asarray(inputs["xyz"], np.float32)
    points = np.asarray(inputs["points"], np.float32)
    W_d1 = np.asarray(inputs["W_d1"], np.float32); b_d1 = np.asarray(inputs["b_d1"], np.float32)
    W_d2 = np.asarray(inputs["W_d2"], np.float32); b_d2 = np.asarray(inputs["b_d2"], np.float32)
    W_g1 = np.asarray(inputs["W_g1"], np.float32); b_g1 = np.asarray(inputs["b_g1"], np.float32)
    W_g2 = np.asarray(inputs["W_g2"], np.float32); b_g2 = np.asarray(inputs["b_g2"], np.float32)
    W_kg = np.asarray(inputs["W_kg"], np.float32)
    W_vg = np.asarray(inputs["W_vg"], np.float32)
    W_q = np.asarray(inputs["W_q"], np.float32)
    W_k = np.asarray(inputs["W_k"], np.float32)
    W_v = np.asarray(inputs["W_v"], np.float32)

    # per-batch global-slot constants
    q_attn = lat_rep @ W_q.T                      # [B, DIM]
    k_g = lat_rep @ W_kg.T
    v_g = lat_rep @ W_vg.T
    tg = q_attn - k_g
    g1g = np.maximum(tg @ W_g1.T + b_g1, 0.0)
    logit_g = g1g @ W_g2.T + b_g2
    exp_g = np.exp(logit_g)                       # [B, DIM]
    egv = exp_g * v_g

    # constants
    qp = np.arange(128)
    esel = (qp[:, None] % 16 == qp[None, :] % 16).astype(np.float16)  # [q',p]
    masks = np.zeros((2, 128, 256), np.float16)
    g_of = qp // 16                               # q' // 16 in 0..7
    for t in range(2):
        for nb in range(16):
            for g in range(16):
                masks[t, :, nb * 16 + g] = (g_of == (g - t * 8)).astype(np.float16)
    ident16 = np.eye(128, dtype=np.float16)
    nident16 = -np.eye(128, dtype=np.float16)
    ident32 = np.eye(128, dtype=np.float32)

    wd1_l = np.concatenate([W_d1.T, b_d1[None, :]], axis=0).astype(np.float16)  # [4,128]

    maps = []
    for core in range(8):
        b, h = core // 2, core % 2
        qsl = slice(h * NQC, (h + 1) * NQC)
        xq = xyz_q[b, qsl]                        # [2048, 3]
        xn = xyz[b]                               # [2048, 3]
        qx4 = np.concatenate([2.0 * xq.T, np.ones((1, NQC), np.float32)], axis=0)
        xt4 = np.concatenate([xn.T, -np.sum(xn * xn, axis=1)[None, :]], axis=0)
        xyzq4 = np.concatenate([xq.T, np.ones((1, NQC), np.float32)], axis=0).astype(np.float16)
        xyzn4 = np.concatenate([xn.T, np.zeros((1, N), np.float32)], axis=0).astype(np.float16)
        pT = points[b].T.astype(np.float16)          # [256, N]
        ptsT = np.concatenate([pT[0:128], pT[128:256]], axis=1)  # [128, 2N]
        colv = np.stack([b_d2, b_g1, b_g2, q_attn[b], exp_g[b], egv[b],
                         b_d2 + q_attn[b]],
                        axis=1).astype(np.float32)
        maps.append({
            "qx4": np.ascontiguousarray(qx4, np.float32),
            "xt4": np.ascontiguousarray(xt4, np.float32),
            "ptsT": np.ascontiguousarray(ptsT),
            "xyzq4": np.ascontiguousarray(xyzq4),
            "xyzn4": np.ascontiguousarray(xyzn4),
            "wk_l": np.ascontiguousarray(np.concatenate(
                [W_k.T[0:128], W_k.T[128:256]], axis=1).astype(np.float16)),
            "wv_l": np.ascontiguousarray(np.concatenate(
                [W_v.T[0:128], W_v.T[128:256]], axis=1).astype(np.float16)),
            "wd1_l": np.ascontiguousarray(wd1_l),
            "wd2_l": np.ascontiguousarray(W_d2.T.astype(np.float16)),
            "wg1_l": np.ascontiguousarray(W_g1.T.astype(np.float16)),
            "wg2_l": np.ascontiguousarray(W_g2.T.astype(np.float16)),
            "colv": np.ascontiguousarray(colv),
            "esel": np.ascontiguousarray(esel),
            "masks": np.ascontiguousarray(
                np.concatenate([masks[0], masks[1]], axis=1)),
            "ident16": ident16,
            "nident16": nident16,
            "ident32": ident32,
        })
    return maps


def kernel(**inputs):
    if "nc" not in _CACHE:
        _CACHE["nc"] = _build()
    nc = _CACHE["nc"]
    maps = _host_prep(inputs)
    res = run_bass_kernel_spmd(nc, maps, core_ids=list(range(8)))
    _CACHE["last_results"] = res
    out = np.empty((B, NQ, DIM), np.float32)
    for core in range(8):
        b, h = core // 2, core % 2
        out[b, h * NQC:(h + 1) * NQC, :] = res.results[core]["out"]
    return out
